# revision 28
# baseline (speedup 1.0000x reference)
"""Causal self-attention (B=4, T=2048, C=1024, 16 heads) on 8 trn2 cores.

Sharding: core c -> (batch b = c//2, head-group g = c%2 of 8 heads).
Each core computes qkv projection for its heads, causal attention, and a
partial c_proj product; the host sums the two partials per batch
(Megatron row-parallel reduce done at gather time).

Kernel layout (per core):
  - host supplies x[b].T (d-major), w slices pre-transposed, all bf16
  - qkv matmuls produce qT/kT d-major [64*2, T] per head-pair and V
    T-major [T, 8 heads, 64(+1 ones col)] for the AV matmul
  - attention computes S.T tiles [k=128 part, q<=512 free] = K Q^T for
    BOTH heads of a pair concurrently (row-tiled: head0 in PE rows
    0-63, head1 in rows 64-127), softmax without max-subtraction
    (S is O(5) so exp is safe), causal mask on diagonal tiles applied
    post-exp as a DVE multiply with a 0/1 bf16 mask
  - AV: out.T[65, q] += [V|1].T @ P.T accumulated over k tiles; row 64
    is the softmax denominator (ones column trick); AV of k-tile pair i
    is emitted during iteration i+1 so STs overlap the previous exp
  - normalize via DVE fast reciprocal (read straight from PSUM) +
    gpsimd partition_broadcast (64 rows) + DVE mul
  - c_proj: y.T = w_projT.T @ attT, partial over this core's channels;
    qkv of the next block runs as PE filler early, proj groups are
    deferred into the late (exp-bound) blocks
  - input DMA is chunked per contraction subtile so the first qkv
    matmul starts ~3us in instead of waiting for the full 3MB load
"""

import math

import numpy as np
import ml_dtypes

B, T, C = 4, 2048, 1024
H = 16
D = 64
P = 128
HL = H // 2          # heads per core
NPAIR = HL // 2      # head pairs per core
KSUB = C // P        # 8 contraction subtiles for qkv
TB = 512             # T block (attention q block, qkv column block)
BF16 = ml_dtypes.bfloat16

SCALE = 1.0 / math.sqrt(D)

_CACHE: dict = {}


def emit_attention(tc, io):
    """Emit the per-core kernel. io maps tensor name -> bass AP.

    Shapes (T_ may be reduced for simulation):
      xT      [C, T_]   bf16   x[b].T
      w_qk    [C, 1024] bf16   columns: [q pair0 | k pair0 | q pair1 | ...]
      w_v     [C, 512]  bf16   v weights for the 8 local heads, head-major
      w_pj    [512, C]  bf16   w_proj[:, local channels].T
      mask01  [128,128] bf16   1 where q >= k (upper incl. in [k,q] layout)
      yT      [C, T_]   bf16   output partial, transposed
    """
    from collections import deque
    from contextlib import ExitStack

    import concourse.mybir as mybir

    nc = tc.nc
    f32 = mybir.dt.float32
    bf = mybir.dt.bfloat16
    EXP = mybir.ActivationFunctionType.Exp

    xT, w_qk, w_v, w_pj = io["xT"], io["w_qk"], io["w_v"], io["w_pj"]
    mask01, yT = io["mask01"], io["yT"]

    T_ = xT.shape[1]
    NTB = T_ // TB       # number of 512-wide T blocks (= q blocks)
    NKT = T_ // P        # number of 128-row k tiles

    xT_r = xT.rearrange("(ko p) t -> p ko t", p=P)      # [128, 8, T]
    wqk_r = w_qk.rearrange("(ko p) n -> p ko n", p=P)   # [128, 8, 1024]
    wv_r = w_v.rearrange("(ko p) n -> p ko n", p=P)     # [128, 8, 512]
    wpj_r = w_pj.rearrange("(ko p) n -> p ko n", p=P)   # [128, 4, 1024]
    yT_r = yT.rearrange("(yt p) t -> p yt t", p=P)      # [128, 8, T]

    marks = []

    def mark(name):
        marks.append((name, nc.next_id()))

    with ExitStack() as ctx:
        const = ctx.enter_context(tc.tile_pool(name="const", bufs=1))
        persist = ctx.enter_context(tc.tile_pool(name="persist", bufs=1))
        work = ctx.enter_context(tc.tile_pool(name="work", bufs=3))
        psum = ctx.enter_context(tc.tile_pool(name="psum", bufs=1, space="PSUM"))

        # ---- constants; wqk + xt0 chunks first so qkv can start early ----
        mark("setup")
        wqk_sb = const.tile([P, KSUB, 2 * HL * D], bf, tag="wqk")
        wv_sb = const.tile([P, KSUB, HL * D], bf, tag="wv")
        wpj_sb = const.tile([P, HL * D // P, C], bf, tag="wpj")
        mask_sb = const.tile([P, P], bf, tag="mask")

        xt0 = work.tile([P, KSUB, TB], bf, tag="xt", bufs=2, name="xt_0")
        for ks in range(KSUB):
            nc.sync.dma_start(wqk_sb[:, ks, :], wqk_r[:, ks, :])
            nc.sync.dma_start(xt0[:, ks, :], xT_r[:, ks, 0:TB])
            if ks == 0:
                nc.sync.dma_start(mask_sb, mask01)
        for ks in range(KSUB):
            nc.sync.dma_start(wv_sb[:, ks, :], wv_r[:, ks, :])
        nc.sync.dma_start(wpj_sb, wpj_r)

        # ---- persistent intermediates ----
        qT_sb = [persist.tile([P, T_], bf, tag=f"qT{p}", name=f"qT{p}")
                 for p in range(NPAIR)]
        kT_sb = [persist.tile([P, T_], bf, tag=f"kT{p}", name=f"kT{p}")
                 for p in range(NPAIR)]
        # V in T-major laid out [1 | 0*63 | v*64] per head so that the AV
        # output's denominator row lands on PSUM partition 0 (where
        # reciprocal_approx_fast can read it) and the v rows span PSUM
        # partitions 64..127 (a >32-partition DVE read must start at 0 or 64).
        # M=128 costs nothing: matmul time is driven by the free dim only.
        VA = 128
        v_aug = persist.tile([P, NKT, HL, VA], bf, tag="vaug")
        nc.gpsimd.memset(v_aug[:, :, :, 0:64], 0.0)
        nc.gpsimd.memset(v_aug[:, :, :, 0], 1.0)
        attT_sb = persist.tile([P, NPAIR, T_], bf, tag="attT")

        # ---- filler work: qkv / proj psum groups fed into attention stalls ----
        # The PE stream is in-order, so exp-wait bubbles inside the attention
        # stretch can only be filled by emitting independent matmul groups
        # between attention units. qkv of the NEXT T block must fully drain
        # before that block's attention; proj of finished blocks can be
        # deferred arbitrarily, so it is held back for the late q blocks
        # where exp dominates.
        qkv_q = deque()
        proj_q = deque()

        # keep-warm fallback: when no real filler is available, emit a small
        # matmul group on always-resident data into a psum tile that is never
        # read. It consumes only otherwise-idle PE time and keeps the PE HAM
        # clock-gate at 8/8 through exp-bound stretches (an idle window
        # re-throttles the PE to 1.2 GHz and then ALL matmuls run ~2x slow).
        dummy_n = [0]

        def emit_dummy():
            ps = psum.tile([P, TB], f32, tag="fps", bufs=2,
                           name=f"warm_{dummy_n[0]}")
            dummy_n[0] += 1
            mark("warm")
            for ks in range(4):
                nc.tensor.matmul(
                    ps,
                    lhsT=wqk_sb[:, 0, ks * P:(ks + 1) * P],
                    rhs=wqk_sb[:, 1, 0:TB],
                    start=(ks == 0), stop=(ks == 3),
                    skip_group_check=True,
                )

        def emit_filler(n=1, allow_dummy=True):
            for _ in range(n):
                if qkv_q:
                    qkv_q.popleft()()
                elif proj_q:
                    proj_q.popleft()()
                elif allow_dummy:
                    emit_dummy()

        def attn_block(p, qb):
            """Attention for head pair p, query block qb (q in [qb*512, qb*512+512))."""
            av = [psum.tile([P, TB], f32, tag="av", bufs=2, name=f"av_{p}_{qb}_{h}")
                  for h in range(2)]
            n_full = 4 * qb

            def emit_av(h, pt, off, i):
                mark("av")
                for j in range(2):
                    kt = i + j
                    nc.tensor.matmul(
                        av[h][:, :],
                        lhsT=v_aug[:, kt, 2 * p + h, :],
                        rhs=pt[:, off + j * TB:off + (j + 1) * TB],
                        start=(kt == 0), stop=False,
                        skip_group_check=True,
                    )

            # full k tiles, processed in pairs per head with head1's stream
            # staggered ONE step behind head0. The two heads' ST matmuls are
            # adjacent in the PE stream and row-tiled (head0 rows 0-63,
            # head1 rows 64-127) so they execute concurrently — and because
            # of the stagger, every PE op emitted at step s consumes an exp
            # that finished a full period earlier, so the in-order PE queue
            # never blocks on ACT.
            nsteps = n_full // 2
            pend_av = [None, None]   # per head: (pt, off, i) awaiting AV
            for s in range(nsteps + 1):
                todo = []            # (h, pair index) STs to emit this step
                if s < nsteps:
                    todo.append((0, s))
                if s >= 1:
                    todo.append((1, s - 1))
                if not todo:
                    continue
                # per-head 2-bank psum tiles (tags st0/st1) keep the
                # exp(s-1,h) -> ST(s,h) -> exp(s,h) chains of the two heads
                # overlapped: while head1's exp runs, head0's next STs fill.
                sts = {}
                mark("stfull")
                for j in range(2):
                    for h, sp in todo:
                        if h not in sts:
                            sts[h] = psum.tile([P, 2 * TB], f32, tag=f"st{h}",
                                               bufs=1, name=f"st_{p}_{qb}_{sp}_{h}")
                        kt = 2 * sp + j
                        d0 = 64 * h
                        nc.tensor.matmul(
                            sts[h][:, j * TB:(j + 1) * TB],
                            lhsT=kT_sb[p][d0:d0 + 64, kt * P:(kt + 1) * P],
                            rhs=qT_sb[p][d0:d0 + 64, qb * TB:(qb + 1) * TB],
                            start=True, stop=True,
                        )
                # filler goes BEFORE the pending AVs: a filler (or keep-warm
                # dummy) then never delays the ST->exp critical chain, and
                # the AVs it delays have a full pipeline period of slack.
                emit_filler(1)
                for h, sp in todo:
                    pt = work.tile([P, 2 * TB], bf, tag=f"pt{h}", bufs=2,
                                   name=f"pt_{p}_{qb}_{sp}_{h}")
                    mark("exp")
                    nc.scalar.activation(pt, sts[h], EXP, scale=SCALE)
                    if pend_av[h] is not None:
                        emit_av(h, *pend_av[h])
                    pend_av[h] = (pt, 0, 2 * sp)
            for h in range(2):
                if pend_av[h] is not None:
                    emit_av(h, *pend_av[h])
                    pend_av[h] = None
            emit_filler(1)

            # diagonal k tiles. Both heads' STs land in one 2-bank psum tile
            # ([128, 2, TB] view) so a single ACT instr computes both exps;
            # the causal mask is a post-exp DVE multiply on the 128x128
            # diagonal subtile (exact: x*0=0, x*1=x in bf16).
            def emit_avd(ptv, j):
                kt = n_full + j
                ncols = TB - j * P
                mark("avd")
                for h in range(2):
                    nc.tensor.matmul(
                        av[h][:, j * P:TB],
                        lhsT=v_aug[:, kt, 2 * p + h, :],
                        rhs=ptv[:, h, 0:ncols],
                        start=(kt == 0), stop=(j == 3),
                        skip_group_check=True,
                    )

            pend_d = None
            for j in range(4):
                kt = n_full + j
                ncols = TB - j * P
                std = psum.tile([P, 2 * TB], f32, tag=f"st{j % 2}", bufs=1,
                                name=f"std_{p}_{qb}_{j}")
                stv = std.rearrange("p (h t) -> p h t", h=2)
                mark("stdiag")
                for h in range(2):
                    d0 = 64 * h
                    nc.tensor.matmul(
                        stv[:, h, 0:ncols],
                        lhsT=kT_sb[p][d0:d0 + 64, kt * P:(kt + 1) * P],
                        rhs=qT_sb[p][d0:d0 + 64, qb * TB + j * P:(qb + 1) * TB],
                        start=True, stop=True,
                    )
                ptd = work.tile([P, 2 * TB], bf, tag="ptd", bufs=2,
                                name=f"ptd_{p}_{qb}_{j}")
                ptv = ptd.rearrange("p (h t) -> p h t", h=2)
                mark("expd")
                nc.scalar.activation(ptv[:, :, 0:ncols], stv[:, :, 0:ncols],
                                     EXP, scale=SCALE)
                mark("maskmul")
                for h in range(2):
                    nc.vector.tensor_mul(
                        out=ptv[:, h, 0:P], in0=ptv[:, h, 0:P], in1=mask_sb)
                if j % 2 == 1:
                    emit_filler(1)
                if pend_d is not None:
                    emit_avd(*pend_d)
                pend_d = (ptv, j)
            emit_avd(*pend_d)

            # normalize by the denominator (AV row 0, on PSUM partition 0
            # where reciprocal_approx_fast can read it directly); the v rows
            # are read straight from PSUM by the DVE multiply.
            for h in range(2):
                mark("norm")
                rcp = work.tile([1, TB], f32, tag="rcp", bufs=2, name=f"rcp_{p}_{qb}_{h}")
                nc.vector.reciprocal_approx_fast(out=rcp, in_=av[h][0:1, :])
                bc = work.tile([64, TB], f32, tag="bc", bufs=2, name=f"bc_{p}_{qb}_{h}")
                nc.gpsimd.partition_broadcast(bc, rcp)
                nc.vector.tensor_mul(
                    out=attT_sb[64 * h:64 * h + 64, p, qb * TB:(qb + 1) * TB],
                    in0=av[h][64:64 + D, :],
                    in1=bc,
                )

        # qkv groups are split into two ~4-matmul halves so a filler pop
        # (~850ns) matches the per-step exp-wait hole; halves of one group
        # share a psum tile via the closure and stay ordered in the deque.
        def qkv_qk_group(xt, tb, wt):
            box = {}

            def go_a():
                box["ps"] = psum.tile([P, TB], f32, tag="fps", bufs=2,
                                      name=f"qk_ps_{tb}_{wt}")
                mark("qkvqk")
                for ks in range(KSUB // 2):
                    nc.tensor.matmul(
                        box["ps"],
                        lhsT=wqk_sb[:, ks, wt * P:(wt + 1) * P],
                        rhs=xt[:, ks, :],
                        start=(ks == 0), stop=False,
                        skip_group_check=True,
                    )

            def go_b():
                mark("qkvqk")
                for ks in range(KSUB // 2, KSUB):
                    nc.tensor.matmul(
                        box["ps"],
                        lhsT=wqk_sb[:, ks, wt * P:(wt + 1) * P],
                        rhs=xt[:, ks, :],
                        start=False, stop=(ks == KSUB - 1),
                        skip_group_check=True,
                    )
                pr, isk = divmod(wt, 2)
                dst = (kT_sb if isk else qT_sb)[pr][:, tb * TB:(tb + 1) * TB]
                mark("qkcopy")
                nc.vector.tensor_copy(out=dst, in_=box["ps"])
            return go_a, go_b

        def qkv_v_group(xt, tb, tt):
            box = {}

            def go_a():
                box["ps"] = psum.tile([P, HL * D], f32, tag="fps", bufs=2,
                                      name=f"v_ps_{tb}_{tt}")
                mark("qkvv")
                for ks in range(KSUB // 2):
                    nc.tensor.matmul(
                        box["ps"],
                        lhsT=xt[:, ks, tt * P:(tt + 1) * P],
                        rhs=wv_sb[:, ks, :],
                        start=(ks == 0), stop=False,
                        skip_group_check=True,
                    )

            def go_b():
                mark("qkvv")
                for ks in range(KSUB // 2, KSUB):
                    nc.tensor.matmul(
                        box["ps"],
                        lhsT=xt[:, ks, tt * P:(tt + 1) * P],
                        rhs=wv_sb[:, ks, :],
                        start=False, stop=(ks == KSUB - 1),
                        skip_group_check=True,
                    )
                kt_idx = tb * (TB // P) + tt
                mark("vcopy")
                nc.vector.tensor_copy(
                    out=v_aug[:, kt_idx, :, 64:64 + D],
                    in_=box["ps"].rearrange("p (h d) -> p h d", h=HL),
                )
            return go_a, go_b

        def proj_group(tb, yrt):
            def go():
                pj = psum.tile([P, TB], f32, tag="fps", bufs=2, name=f"pj_{yrt}_{tb}")
                mark("proj")
                for ks in range(NPAIR):
                    nc.tensor.matmul(
                        pj,
                        lhsT=wpj_sb[:, ks, yrt * P:(yrt + 1) * P],
                        rhs=attT_sb[:, ks, tb * TB:(tb + 1) * TB],
                        start=(ks == 0), stop=(ks == NPAIR - 1),
                    )
                mark("yout")
                yo = work.tile([P, TB], bf, tag="yo", bufs=2, name=f"yo_{yrt}_{tb}")
                nc.vector.tensor_copy(out=yo, in_=pj)
                nc.sync.dma_start(yT_r[:, yrt, tb * TB:(tb + 1) * TB], yo)
            return go

        def emit_xt_dma(tb):
            mark("xtdma")
            xt = work.tile([P, KSUB, TB], bf, tag="xt", bufs=2, name=f"xt_{tb}")
            nc.sync.dma_start(xt, xT_r[:, :, tb * TB:(tb + 1) * TB])
            return xt

        # ---- main loop ----
        # qkv(0) runs up front, ks-major across all 8 psum banks so the PE
        # rides the chunked input DMA (each arriving chunk feeds 8 matmuls);
        # afterwards qkv(tb+1) runs as filler inside attention(tb) and
        # proj(tb') is deferred filler for the late exp-bound blocks.
        st0t = psum.tile([P, 2 * TB], f32, tag="st0", bufs=1, name="qkv0_st0")
        st1t = psum.tile([P, 2 * TB], f32, tag="st1", bufs=1, name="qkv0_st1")
        b0 = [st0t[:, 0:TB], st0t[:, TB:2 * TB],
              st1t[:, 0:TB], st1t[:, TB:2 * TB]] + \
             [psum.tile([P, TB], f32, tag="fps", bufs=2, name="qkv0_f0"),
              psum.tile([P, TB], f32, tag="fps", bufs=2, name="qkv0_f1"),
              psum.tile([P, TB], f32, tag="av", bufs=2, name="qkv0_a0"),
              psum.tile([P, TB], f32, tag="av", bufs=2, name="qkv0_a1")]
        mark("qkvqk")
        for ks in range(KSUB):
            for wt in range(2 * NPAIR):
                nc.tensor.matmul(
                    b0[wt],
                    lhsT=wqk_sb[:, ks, wt * P:(wt + 1) * P],
                    rhs=xt0[:, ks, :],
                    start=(ks == 0), stop=(ks == KSUB - 1),
                    skip_group_check=True,
                )
        mark("qkcopy")
        for wt in range(2 * NPAIR):
            pr, isk = divmod(wt, 2)
            nc.vector.tensor_copy(
                out=(kT_sb if isk else qT_sb)[pr][:, 0:TB], in_=b0[wt])
        for tt in range(TB // P):
            ga, gb = qkv_v_group(xt0, 0, tt)
            ga(); gb()

        for tb in range(NTB):
            if tb + 1 < NTB:
                xt = emit_xt_dma(tb + 1)
                for wt in range(2 * NPAIR):
                    qkv_q.extend(qkv_qk_group(xt, tb + 1, wt))
                for tt in range(TB // P):
                    qkv_q.extend(qkv_v_group(xt, tb + 1, tt))
            if tb - 1 >= 0:
                for yrt in range(C // P):
                    proj_q.append(proj_group(tb - 1, yrt))

            for p in range(NPAIR):
                attn_block(p, qb=tb)
                emit_filler(2, allow_dummy=False)

            # qkv(tb+1) must be in the PE stream before attention(tb+1)
            # consumes it; proj fillers may carry over.
            while qkv_q:
                qkv_q.popleft()()

        while proj_q:
            proj_q.popleft()()

        # final-block proj: contraction split so only the last pair's matmul
        # waits on the final attention block's normalize; the partial sums
        # spread over all 8 now-free psum banks and the copies run on the
        # (idle at this point) scalar engine.
        tbL = NTB - 1
        stf0 = psum.tile([P, 2 * TB], f32, tag="st0", bufs=1, name="pjF_st0")
        stf1 = psum.tile([P, 2 * TB], f32, tag="st1", bufs=1, name="pjF_st1")
        pj_banks = [stf0[:, 0:TB], stf0[:, TB:2 * TB],
                    stf1[:, 0:TB], stf1[:, TB:2 * TB]] + \
                   [psum.tile([P, TB], f32, tag="fps", bufs=2, name="pjF_f0"),
                    psum.tile([P, TB], f32, tag="fps", bufs=2, name="pjF_f1"),
                    psum.tile([P, TB], f32, tag="av", bufs=2, name="pjF_a0"),
                    psum.tile([P, TB], f32, tag="av", bufs=2, name="pjF_a1")]
        mark("proj")
        for yrt in range(C // P):
            for ks in range(NPAIR - 1):
                nc.tensor.matmul(
                    pj_banks[yrt],
                    lhsT=wpj_sb[:, ks, yrt * P:(yrt + 1) * P],
                    rhs=attT_sb[:, ks, tbL * TB:(tbL + 1) * TB],
                    start=(ks == 0), stop=False,
                    skip_group_check=True,
                )
        for yrt in range(C // P):
            nc.tensor.matmul(
                pj_banks[yrt],
                lhsT=wpj_sb[:, NPAIR - 1, yrt * P:(yrt + 1) * P],
                rhs=attT_sb[:, NPAIR - 1, tbL * TB:(tbL + 1) * TB],
                start=False, stop=True,
                skip_group_check=True,
            )
            mark("yout")
            yo = work.tile([P, TB], bf, tag="yo", bufs=2, name=f"yoF_{yrt}")
            nc.scalar.copy(yo, pj_banks[yrt])
            nc.sync.dma_start(yT_r[:, yrt, tbL * TB:(tbL + 1) * TB], yo)

        mark("end")

    return marks


def _build(T_=T):
    if T_ in _CACHE:
        return _CACHE[T_]
    import concourse.bacc as bacc
    import concourse.mybir as mybir
    import concourse.tile as tile

    nc = bacc.Bacc("TRN2", debug=False, num_devices=8)
    bf = mybir.dt.bfloat16
    io = {
        "xT": nc.dram_tensor("xT", [C, T_], bf, kind="ExternalInput").ap(),
        "w_qk": nc.dram_tensor("w_qk", [C, 2 * HL * D], bf, kind="ExternalInput").ap(),
        "w_v": nc.dram_tensor("w_v", [C, HL * D], bf, kind="ExternalInput").ap(),
        "w_pj": nc.dram_tensor("w_pj", [HL * D, C], bf, kind="ExternalInput").ap(),
        "mask01": nc.dram_tensor("mask01", [P, P], bf, kind="ExternalInput").ap(),
        "yT": nc.dram_tensor("yT", [C, T_], bf, kind="ExternalOutput").ap(),
    }
    with tile.TileContext(nc) as tc:
        marks = emit_attention(tc, io)
    try:
        import json
        with open("/tmp/phase_marks.json", "w") as f:
            json.dump(marks, f)
    except Exception:
        pass
    nc.compile()
    _CACHE[T_] = nc
    return nc


def make_core_inputs(x, w_attn, w_proj, core, T_=T):
    """Host-side sharding for one core: (batch, head-group) slice + relayout."""
    b, g = divmod(core, 2)
    gs = slice(g * HL * D, (g + 1) * HL * D)
    q, k, v = w_attn[0:C], w_attn[C:2 * C], w_attn[2 * C:3 * C]
    qg, kg, vg = q[gs], k[gs], v[gs]          # [512, C] each
    blocks = []
    for p in range(NPAIR):
        blocks.append(qg[p * P:(p + 1) * P])
        blocks.append(kg[p * P:(p + 1) * P])
    wqk = np.concatenate(blocks, axis=0).T    # [C, 1024]
    # mask01[k, q] = 1 where q >= k within the diagonal 128x128 subtile
    m01 = np.triu(np.ones((P, P), np.float32))
    return {
        "xT": np.ascontiguousarray(x[b, :T_].T).astype(BF16),
        "w_qk": np.ascontiguousarray(wqk).astype(BF16),
        "w_v": np.ascontiguousarray(vg.T).astype(BF16),
        "w_pj": np.ascontiguousarray(w_proj[:, gs].T).astype(BF16),
        "mask01": m01.astype(BF16),
    }


def kernel(x, w_attn, w_proj):
    x = np.asarray(x, dtype=np.float32)
    w_attn = np.asarray(w_attn, dtype=np.float32)
    w_proj = np.asarray(w_proj, dtype=np.float32)

    from concourse.bass_utils import run_bass_kernel_spmd

    nc = _build()
    in_maps = [make_core_inputs(x, w_attn, w_proj, c) for c in range(8)]
    res = run_bass_kernel_spmd(nc, in_maps, core_ids=list(range(8)))

    y = np.empty((B, T, C), dtype=np.float32)
    for b in range(B):
        yT = (res.results[2 * b]["yT"].astype(np.float32)
              + res.results[2 * b + 1]["yT"].astype(np.float32))
        y[b] = yT.T
    return y


# revision 29
# speedup vs baseline: 1.1877x; 1.1877x over previous
"""Causal self-attention (B=4, T=2048, C=1024, 16 heads) on 8 trn2 cores.

Sharding: core c -> (batch b = c//2, head-group g = c%2 of 8 heads).
Each core computes qkv projection for its heads, causal attention, and a
partial c_proj product; the host sums the two partials per batch
(Megatron row-parallel reduce done at gather time).

Kernel layout (per core):
  - host supplies x[b].T (d-major), w slices pre-transposed, all bf16
  - qkv matmuls produce qT/kT d-major [64*2, T] per head-pair and V
    T-major [T, 8 heads, 64(+1 ones col)] for the AV matmul
  - attention computes S.T tiles [k=128 part, q<=512 free] = K Q^T for
    BOTH heads of a pair concurrently (row-tiled: head0 in PE rows
    0-63, head1 in rows 64-127), softmax without max-subtraction
    (S is O(5) so exp is safe), causal mask on diagonal tiles applied
    post-exp as a DVE multiply with a 0/1 bf16 mask
  - AV: out.T[65, q] += [V|1].T @ P.T accumulated over k tiles; row 64
    is the softmax denominator (ones column trick); AV of k-tile pair i
    is emitted during iteration i+1 so STs overlap the previous exp
  - normalize via DVE fast reciprocal (read straight from PSUM) +
    gpsimd partition_broadcast (64 rows) + DVE mul
  - c_proj: y.T = w_projT.T @ attT, partial over this core's channels;
    qkv of the next block runs as PE filler early, proj groups are
    deferred into the late (exp-bound) blocks
  - input DMA is chunked per contraction subtile so the first qkv
    matmul starts ~3us in instead of waiting for the full 3MB load
"""

import math

import numpy as np
import ml_dtypes

B, T, C = 4, 2048, 1024
H = 16
D = 64
P = 128
HL = H // 2          # heads per core
NPAIR = HL // 2      # head pairs per core
KSUB = C // P        # 8 contraction subtiles for qkv
TB = 512             # T block (attention q block, qkv column block)
BF16 = ml_dtypes.bfloat16

SCALE = 1.0 / math.sqrt(D)

_CACHE: dict = {}


def emit_attention(tc, io):
    """Emit the per-core kernel. io maps tensor name -> bass AP.

    Shapes (T_ may be reduced for simulation):
      xT      [C, T_]   bf16   x[b].T
      w_qk    [C, 1024] bf16   columns: [q pair0 | k pair0 | q pair1 | ...]
      w_v     [C, 512]  bf16   v weights for the 8 local heads, head-major
      w_pj    [512, C]  bf16   w_proj[:, local channels].T
      mask01  [128,128] bf16   1 where q >= k (upper incl. in [k,q] layout)
      yT      [C, T_]   bf16   output partial, transposed
    """
    from collections import deque
    from contextlib import ExitStack

    import concourse.mybir as mybir

    nc = tc.nc
    f32 = mybir.dt.float32
    bf = mybir.dt.bfloat16
    EXP = mybir.ActivationFunctionType.Exp

    xT, w_qk, w_v, w_pj = io["xT"], io["w_qk"], io["w_v"], io["w_pj"]
    mask01, yT = io["mask01"], io["yT"]

    T_ = xT.shape[1]
    NTB = T_ // TB       # number of 512-wide T blocks (= q blocks)
    NKT = T_ // P        # number of 128-row k tiles

    xT_r = xT.rearrange("(ko p) t -> p ko t", p=P)      # [128, 8, T]
    wqk_r = w_qk.rearrange("(ko p) n -> p ko n", p=P)   # [128, 8, 1024]
    wv_r = w_v.rearrange("(ko p) n -> p ko n", p=P)     # [128, 8, 512]
    wpj_r = w_pj.rearrange("(ko p) n -> p ko n", p=P)   # [128, 4, 1024]
    yT_r = yT.rearrange("(yt p) t -> p yt t", p=P)      # [128, 8, T]

    marks = []

    def mark(name):
        marks.append((name, nc.next_id()))

    with ExitStack() as ctx:
        const = ctx.enter_context(tc.tile_pool(name="const", bufs=1))
        persist = ctx.enter_context(tc.tile_pool(name="persist", bufs=1))
        work = ctx.enter_context(tc.tile_pool(name="work", bufs=3))
        psum = ctx.enter_context(tc.tile_pool(name="psum", bufs=1, space="PSUM"))

        # ---- constants; wqk + xt0 chunks first so qkv can start early ----
        mark("setup")
        wqk_sb = const.tile([P, KSUB, 2 * HL * D], bf, tag="wqk")
        wv_sb = const.tile([P, KSUB, HL * D], bf, tag="wv")
        wpj_sb = const.tile([P, HL * D // P, C], bf, tag="wpj")
        mask_sb = const.tile([P, P], bf, tag="mask")

        xt0 = work.tile([P, KSUB, TB], bf, tag="xt", bufs=2, name="xt_0")
        for ks in range(KSUB):
            nc.sync.dma_start(wqk_sb[:, ks, :], wqk_r[:, ks, :])
            nc.sync.dma_start(xt0[:, ks, :], xT_r[:, ks, 0:TB])
            if ks == 0:
                nc.sync.dma_start(mask_sb, mask01)
        for ks in range(KSUB):
            nc.sync.dma_start(wv_sb[:, ks, :], wv_r[:, ks, :])
        nc.sync.dma_start(wpj_sb, wpj_r)

        # ---- persistent intermediates ----
        qT_sb = [persist.tile([P, T_], bf, tag=f"qT{p}", name=f"qT{p}")
                 for p in range(NPAIR)]
        kT_sb = [persist.tile([P, T_], bf, tag=f"kT{p}", name=f"kT{p}")
                 for p in range(NPAIR)]
        # V in T-major laid out [1 | 0*63 | v*64] per head so that the AV
        # output's denominator row lands on PSUM partition 0 (where
        # reciprocal_approx_fast can read it) and the v rows span PSUM
        # partitions 64..127 (a >32-partition DVE read must start at 0 or 64).
        # M=128 costs nothing: matmul time is driven by the free dim only.
        VA = 128
        v_aug = persist.tile([P, NKT, HL, VA], bf, tag="vaug")
        nc.gpsimd.memset(v_aug[:, :, :, 0:64], 0.0)
        nc.gpsimd.memset(v_aug[:, :, :, 0], 1.0)
        attT_sb = persist.tile([P, NPAIR, T_], bf, tag="attT")

        # ---- filler work: qkv / proj psum groups fed into attention stalls ----
        # The PE stream is in-order, so exp-wait bubbles inside the attention
        # stretch can only be filled by emitting independent matmul groups
        # between attention units. qkv of the NEXT T block must fully drain
        # before that block's attention; proj of finished blocks can be
        # deferred arbitrarily, so it is held back for the late q blocks
        # where exp dominates.
        qkv_q = deque()
        proj_q = deque()

        # keep-warm fallback: when no real filler is available, emit a small
        # matmul group on always-resident data into a psum tile that is never
        # read. It consumes only otherwise-idle PE time and keeps the PE HAM
        # clock-gate at 8/8 through exp-bound stretches (an idle window
        # re-throttles the PE to 1.2 GHz and then ALL matmuls run ~2x slow).
        dummy_n = [0]

        def emit_dummy():
            ps = psum.tile([P, TB], f32, tag="fps", bufs=2,
                           name=f"warm_{dummy_n[0]}")
            dummy_n[0] += 1
            mark("warm")
            for ks in range(4):
                nc.tensor.matmul(
                    ps,
                    lhsT=wqk_sb[:, 0, ks * P:(ks + 1) * P],
                    rhs=wqk_sb[:, 1, 0:TB],
                    start=(ks == 0), stop=(ks == 3),
                    skip_group_check=True,
                )

        def emit_filler(n=1, allow_dummy=True):
            for _ in range(n):
                if qkv_q:
                    qkv_q.popleft()()
                elif proj_q:
                    proj_q.popleft()()
                elif allow_dummy:
                    emit_dummy()

        def attn_block(p, qb):
            """Attention for head pair p, query block qb (q in [qb*512, qb*512+512))."""
            av = [psum.tile([P, TB], f32, tag="av", bufs=2, name=f"av_{p}_{qb}_{h}")
                  for h in range(2)]
            n_full = 4 * qb

            def emit_av(h, pt, off, i):
                mark("av")
                for j in range(2):
                    kt = i + j
                    nc.tensor.matmul(
                        av[h][:, :],
                        lhsT=v_aug[:, kt, 2 * p + h, :],
                        rhs=pt[:, off + j * TB:off + (j + 1) * TB],
                        start=(kt == 0), stop=False,
                        skip_group_check=True,
                    )

            # full k tiles, processed in pairs per head with head1's stream
            # staggered ONE step behind head0. The two heads' ST matmuls are
            # adjacent in the PE stream and row-tiled (head0 rows 0-63,
            # head1 rows 64-127) so they execute concurrently — and because
            # of the stagger, every PE op emitted at step s consumes an exp
            # that finished a full period earlier, so the in-order PE queue
            # never blocks on ACT.
            nsteps = n_full // 2
            pend_av = [None, None]   # per head: (pt, off, i) awaiting AV
            for s in range(nsteps + 1):
                todo = []            # (h, pair index) STs to emit this step
                if s < nsteps:
                    todo.append((0, s))
                if s >= 1:
                    todo.append((1, s - 1))
                if not todo:
                    continue
                # per-head 2-bank psum tiles (tags st0/st1) keep the
                # exp(s-1,h) -> ST(s,h) -> exp(s,h) chains of the two heads
                # overlapped: while head1's exp runs, head0's next STs fill.
                sts = {}
                mark("stfull")
                for j in range(2):
                    for h, sp in todo:
                        if h not in sts:
                            sts[h] = psum.tile([P, 2 * TB], f32, tag=f"st{h}",
                                               bufs=1, name=f"st_{p}_{qb}_{sp}_{h}")
                        kt = 2 * sp + j
                        d0 = 64 * h
                        nc.tensor.matmul(
                            sts[h][:, j * TB:(j + 1) * TB],
                            lhsT=kT_sb[p][d0:d0 + 64, kt * P:(kt + 1) * P],
                            rhs=qT_sb[p][d0:d0 + 64, qb * TB:(qb + 1) * TB],
                            start=True, stop=True,
                        )
                # filler goes BEFORE the pending AVs: a filler (or keep-warm
                # dummy) then never delays the ST->exp critical chain, and
                # the AVs it delays have a full pipeline period of slack.
                emit_filler(1)
                for h, sp in todo:
                    pt = work.tile([P, 2 * TB], bf, tag=f"pt{h}", bufs=3,
                                   name=f"pt_{p}_{qb}_{sp}_{h}")
                    mark("exp")
                    nc.scalar.activation(pt, sts[h], EXP, scale=SCALE)
                    if pend_av[h] is not None:
                        emit_av(h, *pend_av[h])
                    pend_av[h] = (pt, 0, 2 * sp)
            for h in range(2):
                if pend_av[h] is not None:
                    emit_av(h, *pend_av[h])
                    pend_av[h] = None
            emit_filler(1)

            # diagonal k tiles. Both heads' STs land in one 2-bank psum tile
            # ([128, 2, TB] view) so a single ACT instr computes both exps;
            # the causal mask is a post-exp DVE multiply on the 128x128
            # diagonal subtile (exact: x*0=0, x*1=x in bf16).
            def emit_avd(ptv, j):
                kt = n_full + j
                ncols = TB - j * P
                mark("avd")
                for h in range(2):
                    nc.tensor.matmul(
                        av[h][:, j * P:TB],
                        lhsT=v_aug[:, kt, 2 * p + h, :],
                        rhs=ptv[:, h, 0:ncols],
                        start=(kt == 0), stop=(j == 3),
                        skip_group_check=True,
                    )

            pend_d = None
            for j in range(4):
                kt = n_full + j
                ncols = TB - j * P
                std = psum.tile([P, 2 * TB], f32, tag=f"st{j % 2}", bufs=1,
                                name=f"std_{p}_{qb}_{j}")
                stv = std.rearrange("p (h t) -> p h t", h=2)
                mark("stdiag")
                for h in range(2):
                    d0 = 64 * h
                    nc.tensor.matmul(
                        stv[:, h, 0:ncols],
                        lhsT=kT_sb[p][d0:d0 + 64, kt * P:(kt + 1) * P],
                        rhs=qT_sb[p][d0:d0 + 64, qb * TB + j * P:(qb + 1) * TB],
                        start=True, stop=True,
                    )
                ptd = work.tile([P, 2 * TB], bf, tag="ptd", bufs=3,
                                name=f"ptd_{p}_{qb}_{j}")
                ptv = ptd.rearrange("p (h t) -> p h t", h=2)
                mark("expd")
                nc.scalar.activation(ptv[:, :, 0:ncols], stv[:, :, 0:ncols],
                                     EXP, scale=SCALE)
                mark("maskmul")
                for h in range(2):
                    nc.vector.tensor_mul(
                        out=ptv[:, h, 0:P], in0=ptv[:, h, 0:P], in1=mask_sb)
                if j % 2 == 1:
                    emit_filler(1)
                if pend_d is not None:
                    emit_avd(*pend_d)
                pend_d = (ptv, j)
            emit_avd(*pend_d)

            # normalize by the denominator (AV row 0, on PSUM partition 0
            # where reciprocal_approx_fast can read it directly); the v rows
            # are read straight from PSUM by the DVE multiply.
            for h in range(2):
                mark("norm")
                rcp = work.tile([1, TB], f32, tag="rcp", bufs=2, name=f"rcp_{p}_{qb}_{h}")
                nc.vector.reciprocal_approx_fast(out=rcp, in_=av[h][0:1, :])
                bc = work.tile([64, TB], f32, tag="bc", bufs=2, name=f"bc_{p}_{qb}_{h}")
                nc.gpsimd.partition_broadcast(bc, rcp)
                nc.vector.tensor_mul(
                    out=attT_sb[64 * h:64 * h + 64, p, qb * TB:(qb + 1) * TB],
                    in0=av[h][64:64 + D, :],
                    in1=bc,
                )

        # qkv groups are split into two ~4-matmul halves so a filler pop
        # (~850ns) matches the per-step exp-wait hole; halves of one group
        # share a psum tile via the closure and stay ordered in the deque.
        def qkv_qk_group(xt, tb, wt):
            box = {}

            def go_a():
                box["ps"] = psum.tile([P, TB], f32, tag="fps", bufs=2,
                                      name=f"qk_ps_{tb}_{wt}")
                mark("qkvqk")
                for ks in range(KSUB // 2):
                    nc.tensor.matmul(
                        box["ps"],
                        lhsT=wqk_sb[:, ks, wt * P:(wt + 1) * P],
                        rhs=xt[:, ks, :],
                        start=(ks == 0), stop=False,
                        skip_group_check=True,
                    )

            def go_b():
                mark("qkvqk")
                for ks in range(KSUB // 2, KSUB):
                    nc.tensor.matmul(
                        box["ps"],
                        lhsT=wqk_sb[:, ks, wt * P:(wt + 1) * P],
                        rhs=xt[:, ks, :],
                        start=False, stop=(ks == KSUB - 1),
                        skip_group_check=True,
                    )
                pr, isk = divmod(wt, 2)
                dst = (kT_sb if isk else qT_sb)[pr][:, tb * TB:(tb + 1) * TB]
                mark("qkcopy")
                nc.vector.tensor_copy(out=dst, in_=box["ps"])
            return go_a, go_b

        def qkv_v_group(xt, tb, tt):
            box = {}

            def go_a():
                box["ps"] = psum.tile([P, HL * D], f32, tag="fps", bufs=2,
                                      name=f"v_ps_{tb}_{tt}")
                mark("qkvv")
                for ks in range(KSUB // 2):
                    nc.tensor.matmul(
                        box["ps"],
                        lhsT=xt[:, ks, tt * P:(tt + 1) * P],
                        rhs=wv_sb[:, ks, :],
                        start=(ks == 0), stop=False,
                        skip_group_check=True,
                    )

            def go_b():
                mark("qkvv")
                for ks in range(KSUB // 2, KSUB):
                    nc.tensor.matmul(
                        box["ps"],
                        lhsT=xt[:, ks, tt * P:(tt + 1) * P],
                        rhs=wv_sb[:, ks, :],
                        start=False, stop=(ks == KSUB - 1),
                        skip_group_check=True,
                    )
                kt_idx = tb * (TB // P) + tt
                mark("vcopy")
                nc.vector.tensor_copy(
                    out=v_aug[:, kt_idx, :, 64:64 + D],
                    in_=box["ps"].rearrange("p (h d) -> p h d", h=HL),
                )
            return go_a, go_b

        def proj_group(tb, yrt):
            def go():
                pj = psum.tile([P, TB], f32, tag="fps", bufs=2, name=f"pj_{yrt}_{tb}")
                mark("proj")
                for ks in range(NPAIR):
                    nc.tensor.matmul(
                        pj,
                        lhsT=wpj_sb[:, ks, yrt * P:(yrt + 1) * P],
                        rhs=attT_sb[:, ks, tb * TB:(tb + 1) * TB],
                        start=(ks == 0), stop=(ks == NPAIR - 1),
                    )
                mark("yout")
                yo = work.tile([P, TB], bf, tag="yo", bufs=2, name=f"yo_{yrt}_{tb}")
                nc.vector.tensor_copy(out=yo, in_=pj)
                nc.sync.dma_start(yT_r[:, yrt, tb * TB:(tb + 1) * TB], yo)
            return go

        def emit_xt_dma(tb):
            mark("xtdma")
            xt = work.tile([P, KSUB, TB], bf, tag="xt", bufs=2, name=f"xt_{tb}")
            nc.sync.dma_start(xt, xT_r[:, :, tb * TB:(tb + 1) * TB])
            return xt

        # ---- main loop ----
        # qkv(0) runs up front, ks-major across all 8 psum banks so the PE
        # rides the chunked input DMA (each arriving chunk feeds 8 matmuls);
        # afterwards qkv(tb+1) runs as filler inside attention(tb) and
        # proj(tb') is deferred filler for the late exp-bound blocks.
        st0t = psum.tile([P, 2 * TB], f32, tag="st0", bufs=1, name="qkv0_st0")
        st1t = psum.tile([P, 2 * TB], f32, tag="st1", bufs=1, name="qkv0_st1")
        b0 = [st0t[:, 0:TB], st0t[:, TB:2 * TB],
              st1t[:, 0:TB], st1t[:, TB:2 * TB]] + \
             [psum.tile([P, TB], f32, tag="fps", bufs=2, name="qkv0_f0"),
              psum.tile([P, TB], f32, tag="fps", bufs=2, name="qkv0_f1"),
              psum.tile([P, TB], f32, tag="av", bufs=2, name="qkv0_a0"),
              psum.tile([P, TB], f32, tag="av", bufs=2, name="qkv0_a1")]
        mark("qkvqk")
        for ks in range(KSUB):
            for wt in range(2 * NPAIR):
                nc.tensor.matmul(
                    b0[wt],
                    lhsT=wqk_sb[:, ks, wt * P:(wt + 1) * P],
                    rhs=xt0[:, ks, :],
                    start=(ks == 0), stop=(ks == KSUB - 1),
                    skip_group_check=True,
                )
        mark("qkcopy")
        for wt in range(2 * NPAIR):
            pr, isk = divmod(wt, 2)
            nc.vector.tensor_copy(
                out=(kT_sb if isk else qT_sb)[pr][:, 0:TB], in_=b0[wt])
        for tt in range(TB // P):
            ga, gb = qkv_v_group(xt0, 0, tt)
            ga(); gb()

        for tb in range(NTB):
            if tb + 1 < NTB:
                xt = emit_xt_dma(tb + 1)
                for wt in range(2 * NPAIR):
                    qkv_q.extend(qkv_qk_group(xt, tb + 1, wt))
                for tt in range(TB // P):
                    qkv_q.extend(qkv_v_group(xt, tb + 1, tt))
            if tb - 1 >= 0:
                for yrt in range(C // P):
                    proj_q.append(proj_group(tb - 1, yrt))

            for p in range(NPAIR):
                attn_block(p, qb=tb)
                emit_filler(2, allow_dummy=False)

            # qkv(tb+1) must be in the PE stream before attention(tb+1)
            # consumes it; proj fillers may carry over.
            while qkv_q:
                qkv_q.popleft()()

        while proj_q:
            proj_q.popleft()()

        # final-block proj: contraction split so only the last pair's matmul
        # waits on the final attention block's normalize; the partial sums
        # spread over all 8 now-free psum banks and the copies run on the
        # (idle at this point) scalar engine.
        tbL = NTB - 1
        stf0 = psum.tile([P, 2 * TB], f32, tag="st0", bufs=1, name="pjF_st0")
        stf1 = psum.tile([P, 2 * TB], f32, tag="st1", bufs=1, name="pjF_st1")
        pj_banks = [stf0[:, 0:TB], stf0[:, TB:2 * TB],
                    stf1[:, 0:TB], stf1[:, TB:2 * TB]] + \
                   [psum.tile([P, TB], f32, tag="fps", bufs=2, name="pjF_f0"),
                    psum.tile([P, TB], f32, tag="fps", bufs=2, name="pjF_f1"),
                    psum.tile([P, TB], f32, tag="av", bufs=2, name="pjF_a0"),
                    psum.tile([P, TB], f32, tag="av", bufs=2, name="pjF_a1")]
        mark("proj")
        for yrt in range(C // P):
            for ks in range(NPAIR - 1):
                nc.tensor.matmul(
                    pj_banks[yrt],
                    lhsT=wpj_sb[:, ks, yrt * P:(yrt + 1) * P],
                    rhs=attT_sb[:, ks, tbL * TB:(tbL + 1) * TB],
                    start=(ks == 0), stop=False,
                    skip_group_check=True,
                )
        for yrt in range(C // P):
            nc.tensor.matmul(
                pj_banks[yrt],
                lhsT=wpj_sb[:, NPAIR - 1, yrt * P:(yrt + 1) * P],
                rhs=attT_sb[:, NPAIR - 1, tbL * TB:(tbL + 1) * TB],
                start=False, stop=True,
                skip_group_check=True,
            )
            mark("yout")
            yo = work.tile([P, TB], bf, tag="yo", bufs=2, name=f"yoF_{yrt}")
            nc.scalar.copy(yo, pj_banks[yrt])
            nc.sync.dma_start(yT_r[:, yrt, tbL * TB:(tbL + 1) * TB], yo)

        mark("end")

    return marks


def _build(T_=T):
    if T_ in _CACHE:
        return _CACHE[T_]
    import concourse.bacc as bacc
    import concourse.mybir as mybir
    import concourse.tile as tile

    nc = bacc.Bacc("TRN2", debug=False, num_devices=8)
    bf = mybir.dt.bfloat16
    io = {
        "xT": nc.dram_tensor("xT", [C, T_], bf, kind="ExternalInput").ap(),
        "w_qk": nc.dram_tensor("w_qk", [C, 2 * HL * D], bf, kind="ExternalInput").ap(),
        "w_v": nc.dram_tensor("w_v", [C, HL * D], bf, kind="ExternalInput").ap(),
        "w_pj": nc.dram_tensor("w_pj", [HL * D, C], bf, kind="ExternalInput").ap(),
        "mask01": nc.dram_tensor("mask01", [P, P], bf, kind="ExternalInput").ap(),
        "yT": nc.dram_tensor("yT", [C, T_], bf, kind="ExternalOutput").ap(),
    }
    with tile.TileContext(nc) as tc:
        marks = emit_attention(tc, io)
    try:
        import json
        with open("/tmp/phase_marks.json", "w") as f:
            json.dump(marks, f)
    except Exception:
        pass
    nc.compile()
    _CACHE[T_] = nc
    return nc


def make_core_inputs(x, w_attn, w_proj, core, T_=T):
    """Host-side sharding for one core: (batch, head-group) slice + relayout."""
    b, g = divmod(core, 2)
    gs = slice(g * HL * D, (g + 1) * HL * D)
    q, k, v = w_attn[0:C], w_attn[C:2 * C], w_attn[2 * C:3 * C]
    qg, kg, vg = q[gs], k[gs], v[gs]          # [512, C] each
    blocks = []
    for p in range(NPAIR):
        blocks.append(qg[p * P:(p + 1) * P])
        blocks.append(kg[p * P:(p + 1) * P])
    wqk = np.concatenate(blocks, axis=0).T    # [C, 1024]
    # mask01[k, q] = 1 where q >= k within the diagonal 128x128 subtile
    m01 = np.triu(np.ones((P, P), np.float32))
    return {
        "xT": np.ascontiguousarray(x[b, :T_].T).astype(BF16),
        "w_qk": np.ascontiguousarray(wqk).astype(BF16),
        "w_v": np.ascontiguousarray(vg.T).astype(BF16),
        "w_pj": np.ascontiguousarray(w_proj[:, gs].T).astype(BF16),
        "mask01": m01.astype(BF16),
    }


def kernel(x, w_attn, w_proj):
    x = np.asarray(x, dtype=np.float32)
    w_attn = np.asarray(w_attn, dtype=np.float32)
    w_proj = np.asarray(w_proj, dtype=np.float32)

    from concourse.bass_utils import run_bass_kernel_spmd

    nc = _build()
    in_maps = [make_core_inputs(x, w_attn, w_proj, c) for c in range(8)]
    res = run_bass_kernel_spmd(nc, in_maps, core_ids=list(range(8)))

    y = np.empty((B, T, C), dtype=np.float32)
    for b in range(B):
        yT = (res.results[2 * b]["yT"].astype(np.float32)
              + res.results[2 * b + 1]["yT"].astype(np.float32))
        y[b] = yT.T
    return y


# revision 33
# speedup vs baseline: 1.2050x; 1.0146x over previous
"""Causal self-attention (B=4, T=2048, C=1024, 16 heads) on 8 trn2 cores.

Sharding: core c -> (batch b = c//2, head-group g = c%2 of 8 heads).
Each core computes qkv projection for its heads, causal attention, and a
partial c_proj product; the host sums the two partials per batch
(Megatron row-parallel reduce done at gather time).

Kernel layout (per core):
  - host supplies x[b].T (d-major), w slices pre-transposed, all bf16
  - qkv matmuls produce qT/kT d-major [64*2, T] per head-pair and V
    T-major [T, 8 heads, 64(+1 ones col)] for the AV matmul
  - attention computes S.T tiles [k=128 part, q<=512 free] = K Q^T for
    BOTH heads of a pair concurrently (row-tiled: head0 in PE rows
    0-63, head1 in rows 64-127), softmax without max-subtraction
    (S is O(5) so exp is safe), causal mask on diagonal tiles applied
    post-exp as a DVE multiply with a 0/1 bf16 mask
  - AV: out.T[65, q] += [V|1].T @ P.T accumulated over k tiles; row 64
    is the softmax denominator (ones column trick); AV of k-tile pair i
    is emitted during iteration i+1 so STs overlap the previous exp
  - normalize via DVE fast reciprocal (read straight from PSUM) +
    gpsimd partition_broadcast (64 rows) + DVE mul
  - c_proj: y.T = w_projT.T @ attT, partial over this core's channels;
    qkv of the next block runs as PE filler early, proj groups are
    deferred into the late (exp-bound) blocks
  - input DMA is chunked per contraction subtile so the first qkv
    matmul starts ~3us in instead of waiting for the full 3MB load
"""

import math

import numpy as np
import ml_dtypes

B, T, C = 4, 2048, 1024
H = 16
D = 64
P = 128
HL = H // 2          # heads per core
NPAIR = HL // 2      # head pairs per core
KSUB = C // P        # 8 contraction subtiles for qkv
TB = 512             # T block (attention q block, qkv column block)
BF16 = ml_dtypes.bfloat16

SCALE = 1.0 / math.sqrt(D)

_CACHE: dict = {}


def emit_attention(tc, io):
    """Emit the per-core kernel. io maps tensor name -> bass AP.

    Shapes (T_ may be reduced for simulation):
      xT      [C, T_]   bf16   x[b].T
      w_qk    [C, 1024] bf16   columns: [q pair0 | k pair0 | q pair1 | ...]
      w_v     [C, 512]  bf16   v weights for the 8 local heads, head-major
      w_pj    [512, C]  bf16   w_proj[:, local channels].T
      mask01  [128,128] bf16   1 where q >= k (upper incl. in [k,q] layout)
      yT      [C, T_]   bf16   output partial, transposed
    """
    from collections import deque
    from contextlib import ExitStack

    import concourse.mybir as mybir

    nc = tc.nc
    f32 = mybir.dt.float32
    bf = mybir.dt.bfloat16
    EXP = mybir.ActivationFunctionType.Exp

    xT, w_qk, w_v, w_pj = io["xT"], io["w_qk"], io["w_v"], io["w_pj"]
    mask01, yT = io["mask01"], io["yT"]

    T_ = xT.shape[1]
    NTB = T_ // TB       # number of 512-wide T blocks (= q blocks)
    NKT = T_ // P        # number of 128-row k tiles

    xT_r = xT.rearrange("(ko p) t -> p ko t", p=P)      # [128, 8, T]
    wqk_r = w_qk.rearrange("(ko p) n -> p ko n", p=P)   # [128, 8, 1024]
    wv_r = w_v.rearrange("(ko p) n -> p ko n", p=P)     # [128, 8, 512]
    wpj_r = w_pj.rearrange("(ko p) n -> p ko n", p=P)   # [128, 4, 1024]
    yT_r = yT.rearrange("(yt p) t -> p yt t", p=P)      # [128, 8, T]

    marks = []

    def mark(name):
        marks.append((name, nc.next_id()))

    with ExitStack() as ctx:
        const = ctx.enter_context(tc.tile_pool(name="const", bufs=1))
        persist = ctx.enter_context(tc.tile_pool(name="persist", bufs=1))
        work = ctx.enter_context(tc.tile_pool(name="work", bufs=3))
        psum = ctx.enter_context(tc.tile_pool(name="psum", bufs=1, space="PSUM"))

        # ---- constants; wqk + xt0 chunks first so qkv can start early ----
        mark("setup")
        wqk_sb = const.tile([P, KSUB, 2 * HL * D], bf, tag="wqk")
        wv_sb = const.tile([P, KSUB, HL * D], bf, tag="wv")
        wpj_sb = const.tile([P, HL * D // P, C], bf, tag="wpj")
        mask_sb = const.tile([P, P], bf, tag="mask")

        xt0 = work.tile([P, KSUB, TB], bf, tag="xt", bufs=2, name="xt_0")
        for ks in range(KSUB):
            nc.sync.dma_start(wqk_sb[:, ks, :], wqk_r[:, ks, :])
            nc.sync.dma_start(xt0[:, ks, :], xT_r[:, ks, 0:TB])
            if ks == 0:
                nc.sync.dma_start(mask_sb, mask01)
        for ks in range(KSUB):
            nc.sync.dma_start(wv_sb[:, ks, :], wv_r[:, ks, :])
        nc.sync.dma_start(wpj_sb, wpj_r)

        # ---- persistent intermediates ----
        qT_sb = [persist.tile([P, T_], bf, tag=f"qT{p}", name=f"qT{p}")
                 for p in range(NPAIR)]
        kT_sb = [persist.tile([P, T_], bf, tag=f"kT{p}", name=f"kT{p}")
                 for p in range(NPAIR)]
        # V in T-major laid out [1 | 0*63 | v*64] per head so that the AV
        # output's denominator row lands on PSUM partition 0 (where
        # reciprocal_approx_fast can read it) and the v rows span PSUM
        # partitions 64..127 (a >32-partition DVE read must start at 0 or 64).
        # M=128 costs nothing: matmul time is driven by the free dim only.
        VA = 128
        v_aug = persist.tile([P, NKT, HL, VA], bf, tag="vaug")
        nc.gpsimd.memset(v_aug[:, :, :, 0:64], 0.0)
        nc.gpsimd.memset(v_aug[:, :, :, 0], 1.0)
        attT_sb = persist.tile([P, NPAIR, T_], bf, tag="attT")

        # ---- filler work: qkv / proj psum groups fed into attention stalls ----
        # The PE stream is in-order, so exp-wait bubbles inside the attention
        # stretch can only be filled by emitting independent matmul groups
        # between attention units. qkv of the NEXT T block must fully drain
        # before that block's attention; proj of finished blocks can be
        # deferred arbitrarily, so it is held back for the late q blocks
        # where exp dominates.
        qkv_q = deque()
        proj_q = deque()

        # keep-warm fallback: when no real filler is available, emit a small
        # matmul group on always-resident data into a psum tile that is never
        # read. It consumes only otherwise-idle PE time and keeps the PE HAM
        # clock-gate at 8/8 through exp-bound stretches (an idle window
        # re-throttles the PE to 1.2 GHz and then ALL matmuls run ~2x slow).
        dummy_n = [0]

        def emit_dummy():
            ps = psum.tile([P, TB], f32, tag="fps", bufs=2,
                           name=f"warm_{dummy_n[0]}")
            dummy_n[0] += 1
            mark("warm")
            for ks in range(4):
                nc.tensor.matmul(
                    ps,
                    lhsT=wqk_sb[:, 0, ks * P:(ks + 1) * P],
                    rhs=wqk_sb[:, 1, 0:TB],
                    start=(ks == 0), stop=(ks == 3),
                    skip_group_check=True,
                )

        def emit_filler(n=1, allow_dummy=True):
            for _ in range(n):
                if qkv_q:
                    qkv_q.popleft()[1]()
                elif proj_q:
                    proj_q.popleft()()
                elif allow_dummy:
                    emit_dummy()

        def drain_keys(keys):
            """Force-emit queued qkv groups matching `keys`, keep the rest.

            Lets qkv(tb) slide INTO attention(tb): attn_block(p, tb) only
            needs its own pair's q/k groups before it starts and v(tb)
            before its diagonal phase, so the later pairs' groups stay in
            the queue and fill exp-wait holes inside the block itself.
            """
            keep = deque()
            while qkv_q:
                key, fn = qkv_q.popleft()
                if key in keys:
                    fn()
                else:
                    keep.append((key, fn))
            qkv_q.extend(keep)

        def attn_block(p, qb, pre_diag=None):
            """Attention for head pair p, query block qb (q in [qb*512, qb*512+512))."""
            av = [psum.tile([P, TB], f32, tag="av", bufs=2, name=f"av_{p}_{qb}_{h}")
                  for h in range(2)]
            n_full = 4 * qb

            def emit_av(h, pt, off, i):
                mark("av")
                for j in range(2):
                    kt = i + j
                    nc.tensor.matmul(
                        av[h][:, :],
                        lhsT=v_aug[:, kt, 2 * p + h, :],
                        rhs=pt[:, off + j * TB:off + (j + 1) * TB],
                        start=(kt == 0), stop=False,
                        skip_group_check=True,
                    )

            # full k tiles, processed in pairs per head with head1's stream
            # staggered ONE step behind head0. The two heads' ST matmuls are
            # adjacent in the PE stream and row-tiled (head0 rows 0-63,
            # head1 rows 64-127) so they execute concurrently — and because
            # of the stagger, every PE op emitted at step s consumes an exp
            # that finished a full period earlier, so the in-order PE queue
            # never blocks on ACT.
            nsteps = n_full // 2
            pend_av = [None, None]   # per head: (pt, off, i) awaiting AV
            for s in range(nsteps + 1):
                todo = []            # (h, pair index) STs to emit this step
                if s < nsteps:
                    todo.append((0, s))
                if s >= 1:
                    todo.append((1, s - 1))
                if not todo:
                    continue
                # per-head 2-bank psum tiles (tags st0/st1) keep the
                # exp(s-1,h) -> ST(s,h) -> exp(s,h) chains of the two heads
                # overlapped: while head1's exp runs, head0's next STs fill.
                sts = {}
                mark("stfull")
                for j in range(2):
                    for h, sp in todo:
                        if h not in sts:
                            sts[h] = psum.tile([P, 2 * TB], f32, tag=f"st{h}",
                                               bufs=1, name=f"st_{p}_{qb}_{sp}_{h}")
                        kt = 2 * sp + j
                        d0 = 64 * h
                        nc.tensor.matmul(
                            sts[h][:, j * TB:(j + 1) * TB],
                            lhsT=kT_sb[p][d0:d0 + 64, kt * P:(kt + 1) * P],
                            rhs=qT_sb[p][d0:d0 + 64, qb * TB:(qb + 1) * TB],
                            start=True, stop=True,
                        )
                # filler goes BEFORE the pending AVs: a filler (or keep-warm
                # dummy) then never delays the ST->exp critical chain, and
                # the AVs it delays have a full pipeline period of slack.
                emit_filler(1)
                for h, sp in todo:
                    pt = work.tile([P, 2 * TB], bf, tag=f"pt{h}", bufs=3,
                                   name=f"pt_{p}_{qb}_{sp}_{h}")
                    mark("exp")
                    nc.scalar.activation(pt, sts[h], EXP, scale=SCALE)
                    if pend_av[h] is not None:
                        emit_av(h, *pend_av[h])
                    pend_av[h] = (pt, 0, 2 * sp)
            for h in range(2):
                if pend_av[h] is not None:
                    emit_av(h, *pend_av[h])
                    pend_av[h] = None
            emit_filler(1)

            # diagonal k tiles. Both heads' STs land in one 2-bank psum tile
            # ([128, 2, TB] view) so a single ACT instr computes both exps;
            # the causal mask is a post-exp DVE multiply on the 128x128
            # diagonal subtile (exact: x*0=0, x*1=x in bf16).
            if pre_diag is not None:
                pre_diag()

            def emit_avd(ptv, j):
                kt = n_full + j
                ncols = TB - j * P
                mark("avd")
                for h in range(2):
                    nc.tensor.matmul(
                        av[h][:, j * P:TB],
                        lhsT=v_aug[:, kt, 2 * p + h, :],
                        rhs=ptv[:, h, 0:ncols],
                        start=(kt == 0), stop=(j == 3),
                        skip_group_check=True,
                    )

            pend_d = None
            for j in range(4):
                kt = n_full + j
                ncols = TB - j * P
                std = psum.tile([P, 2 * TB], f32, tag=f"st{j % 2}", bufs=1,
                                name=f"std_{p}_{qb}_{j}")
                stv = std.rearrange("p (h t) -> p h t", h=2)
                mark("stdiag")
                for h in range(2):
                    d0 = 64 * h
                    nc.tensor.matmul(
                        stv[:, h, 0:ncols],
                        lhsT=kT_sb[p][d0:d0 + 64, kt * P:(kt + 1) * P],
                        rhs=qT_sb[p][d0:d0 + 64, qb * TB + j * P:(qb + 1) * TB],
                        start=True, stop=True,
                    )
                ptd = work.tile([P, 2 * TB], bf, tag="ptd", bufs=3,
                                name=f"ptd_{p}_{qb}_{j}")
                ptv = ptd.rearrange("p (h t) -> p h t", h=2)
                mark("expd")
                nc.scalar.activation(ptv[:, :, 0:ncols], stv[:, :, 0:ncols],
                                     EXP, scale=SCALE)
                mark("maskmul")
                for h in range(2):
                    nc.vector.tensor_mul(
                        out=ptv[:, h, 0:P], in0=ptv[:, h, 0:P], in1=mask_sb)
                if j % 2 == 1:
                    emit_filler(1)
                if pend_d is not None:
                    emit_avd(*pend_d)
                pend_d = (ptv, j)
            emit_avd(*pend_d)

            # normalize by the denominator (AV row 0, on PSUM partition 0
            # where reciprocal_approx_fast can read it directly); the v rows
            # are read straight from PSUM by the DVE multiply.
            for h in range(2):
                mark("norm")
                rcp = work.tile([1, TB], f32, tag="rcp", bufs=2, name=f"rcp_{p}_{qb}_{h}")
                nc.vector.reciprocal_approx_fast(out=rcp, in_=av[h][0:1, :])
                bc = work.tile([64, TB], f32, tag="bc", bufs=2, name=f"bc_{p}_{qb}_{h}")
                nc.gpsimd.partition_broadcast(bc, rcp)
                nc.vector.tensor_mul(
                    out=attT_sb[64 * h:64 * h + 64, p, qb * TB:(qb + 1) * TB],
                    in0=av[h][64:64 + D, :],
                    in1=bc,
                )

        # qkv groups are split into two ~4-matmul halves so a filler pop
        # (~850ns) matches the per-step exp-wait hole; halves of one group
        # share a psum tile via the closure and stay ordered in the deque.
        def qkv_qk_group(xt, tb, wt):
            box = {}

            def go_a():
                box["ps"] = psum.tile([P, TB], f32, tag="fps", bufs=2,
                                      name=f"qk_ps_{tb}_{wt}")
                mark("qkvqk")
                for ks in range(KSUB // 2):
                    nc.tensor.matmul(
                        box["ps"],
                        lhsT=wqk_sb[:, ks, wt * P:(wt + 1) * P],
                        rhs=xt[:, ks, :],
                        start=(ks == 0), stop=False,
                        skip_group_check=True,
                    )

            def go_b():
                mark("qkvqk")
                for ks in range(KSUB // 2, KSUB):
                    nc.tensor.matmul(
                        box["ps"],
                        lhsT=wqk_sb[:, ks, wt * P:(wt + 1) * P],
                        rhs=xt[:, ks, :],
                        start=False, stop=(ks == KSUB - 1),
                        skip_group_check=True,
                    )
                pr, isk = divmod(wt, 2)
                dst = (kT_sb if isk else qT_sb)[pr][:, tb * TB:(tb + 1) * TB]
                mark("qkcopy")
                nc.vector.tensor_copy(out=dst, in_=box["ps"])
            return go_a, go_b

        def qkv_v_group(xt, tb, tt):
            box = {}

            def go_a():
                box["ps"] = psum.tile([P, HL * D], f32, tag="fps", bufs=2,
                                      name=f"v_ps_{tb}_{tt}")
                mark("qkvv")
                for ks in range(KSUB // 2):
                    nc.tensor.matmul(
                        box["ps"],
                        lhsT=xt[:, ks, tt * P:(tt + 1) * P],
                        rhs=wv_sb[:, ks, :],
                        start=(ks == 0), stop=False,
                        skip_group_check=True,
                    )

            def go_b():
                mark("qkvv")
                for ks in range(KSUB // 2, KSUB):
                    nc.tensor.matmul(
                        box["ps"],
                        lhsT=xt[:, ks, tt * P:(tt + 1) * P],
                        rhs=wv_sb[:, ks, :],
                        start=False, stop=(ks == KSUB - 1),
                        skip_group_check=True,
                    )
                kt_idx = tb * (TB // P) + tt
                mark("vcopy")
                nc.vector.tensor_copy(
                    out=v_aug[:, kt_idx, :, 64:64 + D],
                    in_=box["ps"].rearrange("p (h d) -> p h d", h=HL),
                )
            return go_a, go_b

        def proj_group(tb, yrt):
            def go():
                pj = psum.tile([P, TB], f32, tag="fps", bufs=2, name=f"pj_{yrt}_{tb}")
                mark("proj")
                for ks in range(NPAIR):
                    nc.tensor.matmul(
                        pj,
                        lhsT=wpj_sb[:, ks, yrt * P:(yrt + 1) * P],
                        rhs=attT_sb[:, ks, tb * TB:(tb + 1) * TB],
                        start=(ks == 0), stop=(ks == NPAIR - 1),
                    )
                mark("yout")
                yo = work.tile([P, TB], bf, tag="yo", bufs=2, name=f"yo_{yrt}_{tb}")
                nc.vector.tensor_copy(out=yo, in_=pj)
                nc.sync.dma_start(yT_r[:, yrt, tb * TB:(tb + 1) * TB], yo)
            return go

        def emit_xt_dma(tb):
            mark("xtdma")
            xt = work.tile([P, KSUB, TB], bf, tag="xt", bufs=2, name=f"xt_{tb}")
            nc.sync.dma_start(xt, xT_r[:, :, tb * TB:(tb + 1) * TB])
            return xt

        # ---- main loop ----
        # qkv(0) runs up front, ks-major across all 8 psum banks so the PE
        # rides the chunked input DMA (each arriving chunk feeds 8 matmuls);
        # afterwards qkv(tb+1) runs as filler inside attention(tb) and
        # proj(tb') is deferred filler for the late exp-bound blocks.
        st0t = psum.tile([P, 2 * TB], f32, tag="st0", bufs=1, name="qkv0_st0")
        st1t = psum.tile([P, 2 * TB], f32, tag="st1", bufs=1, name="qkv0_st1")
        b0 = [st0t[:, 0:TB], st0t[:, TB:2 * TB],
              st1t[:, 0:TB], st1t[:, TB:2 * TB]] + \
             [psum.tile([P, TB], f32, tag="fps", bufs=2, name="qkv0_f0"),
              psum.tile([P, TB], f32, tag="fps", bufs=2, name="qkv0_f1"),
              psum.tile([P, TB], f32, tag="av", bufs=2, name="qkv0_a0"),
              psum.tile([P, TB], f32, tag="av", bufs=2, name="qkv0_a1")]
        mark("qkvqk")
        for ks in range(KSUB):
            for wt in range(2 * NPAIR):
                nc.tensor.matmul(
                    b0[wt],
                    lhsT=wqk_sb[:, ks, wt * P:(wt + 1) * P],
                    rhs=xt0[:, ks, :],
                    start=(ks == 0), stop=(ks == KSUB - 1),
                    skip_group_check=True,
                )
        mark("qkcopy")
        for wt in range(2 * NPAIR):
            pr, isk = divmod(wt, 2)
            nc.vector.tensor_copy(
                out=(kT_sb if isk else qT_sb)[pr][:, 0:TB], in_=b0[wt])
        for tt in range(TB // P):
            ga, gb = qkv_v_group(xt0, 0, tt)
            ga(); gb()

        for tb in range(NTB):
            if tb + 1 < NTB:
                xt = emit_xt_dma(tb + 1)
                for wt in range(2 * NPAIR):
                    for g in qkv_qk_group(xt, tb + 1, wt):
                        qkv_q.append((("qk", tb + 1, wt // 2), g))
                for tt in range(TB // P):
                    for g in qkv_v_group(xt, tb + 1, tt):
                        qkv_q.append((("v", tb + 1), g))
            if tb - 1 >= 0:
                for yrt in range(C // P):
                    proj_q.append(proj_group(tb - 1, yrt))

            for p in range(NPAIR):
                drain_keys({("qk", tb, p)})
                attn_block(p, qb=tb,
                           pre_diag=lambda: drain_keys({("v", tb)}))
                emit_filler(2, allow_dummy=False)

        while proj_q:
            proj_q.popleft()()

        # final-block proj: contraction split so only the last pair's matmul
        # waits on the final attention block's normalize; the partial sums
        # spread over all 8 now-free psum banks and the copies run on the
        # (idle at this point) scalar engine.
        tbL = NTB - 1
        stf0 = psum.tile([P, 2 * TB], f32, tag="st0", bufs=1, name="pjF_st0")
        stf1 = psum.tile([P, 2 * TB], f32, tag="st1", bufs=1, name="pjF_st1")
        pj_banks = [stf0[:, 0:TB], stf0[:, TB:2 * TB],
                    stf1[:, 0:TB], stf1[:, TB:2 * TB]] + \
                   [psum.tile([P, TB], f32, tag="fps", bufs=2, name="pjF_f0"),
                    psum.tile([P, TB], f32, tag="fps", bufs=2, name="pjF_f1"),
                    psum.tile([P, TB], f32, tag="av", bufs=2, name="pjF_a0"),
                    psum.tile([P, TB], f32, tag="av", bufs=2, name="pjF_a1")]
        mark("proj")
        for yrt in range(C // P):
            for ks in range(NPAIR - 1):
                nc.tensor.matmul(
                    pj_banks[yrt],
                    lhsT=wpj_sb[:, ks, yrt * P:(yrt + 1) * P],
                    rhs=attT_sb[:, ks, tbL * TB:(tbL + 1) * TB],
                    start=(ks == 0), stop=False,
                    skip_group_check=True,
                )
        for yrt in range(C // P):
            nc.tensor.matmul(
                pj_banks[yrt],
                lhsT=wpj_sb[:, NPAIR - 1, yrt * P:(yrt + 1) * P],
                rhs=attT_sb[:, NPAIR - 1, tbL * TB:(tbL + 1) * TB],
                start=False, stop=True,
                skip_group_check=True,
            )
            mark("yout")
            yo = work.tile([P, TB], bf, tag="yo", bufs=2, name=f"yoF_{yrt}")
            nc.scalar.copy(yo, pj_banks[yrt])
            nc.sync.dma_start(yT_r[:, yrt, tbL * TB:(tbL + 1) * TB], yo)

        mark("end")

    return marks


def _build(T_=T):
    if T_ in _CACHE:
        return _CACHE[T_]
    import concourse.bacc as bacc
    import concourse.mybir as mybir
    import concourse.tile as tile

    nc = bacc.Bacc("TRN2", debug=False, num_devices=8)
    bf = mybir.dt.bfloat16
    io = {
        "xT": nc.dram_tensor("xT", [C, T_], bf, kind="ExternalInput").ap(),
        "w_qk": nc.dram_tensor("w_qk", [C, 2 * HL * D], bf, kind="ExternalInput").ap(),
        "w_v": nc.dram_tensor("w_v", [C, HL * D], bf, kind="ExternalInput").ap(),
        "w_pj": nc.dram_tensor("w_pj", [HL * D, C], bf, kind="ExternalInput").ap(),
        "mask01": nc.dram_tensor("mask01", [P, P], bf, kind="ExternalInput").ap(),
        "yT": nc.dram_tensor("yT", [C, T_], bf, kind="ExternalOutput").ap(),
    }
    with tile.TileContext(nc) as tc:
        marks = emit_attention(tc, io)
    try:
        import json
        with open("/tmp/phase_marks.json", "w") as f:
            json.dump(marks, f)
    except Exception:
        pass
    nc.compile()
    _CACHE[T_] = nc
    return nc


def make_core_inputs(x, w_attn, w_proj, core, T_=T):
    """Host-side sharding for one core: (batch, head-group) slice + relayout."""
    b, g = divmod(core, 2)
    gs = slice(g * HL * D, (g + 1) * HL * D)
    q, k, v = w_attn[0:C], w_attn[C:2 * C], w_attn[2 * C:3 * C]
    qg, kg, vg = q[gs], k[gs], v[gs]          # [512, C] each
    blocks = []
    for p in range(NPAIR):
        blocks.append(qg[p * P:(p + 1) * P])
        blocks.append(kg[p * P:(p + 1) * P])
    wqk = np.concatenate(blocks, axis=0).T    # [C, 1024]
    # mask01[k, q] = 1 where q >= k within the diagonal 128x128 subtile
    m01 = np.triu(np.ones((P, P), np.float32))
    return {
        "xT": np.ascontiguousarray(x[b, :T_].T).astype(BF16),
        "w_qk": np.ascontiguousarray(wqk).astype(BF16),
        "w_v": np.ascontiguousarray(vg.T).astype(BF16),
        "w_pj": np.ascontiguousarray(w_proj[:, gs].T).astype(BF16),
        "mask01": m01.astype(BF16),
    }


def kernel(x, w_attn, w_proj):
    x = np.asarray(x, dtype=np.float32)
    w_attn = np.asarray(w_attn, dtype=np.float32)
    w_proj = np.asarray(w_proj, dtype=np.float32)

    from concourse.bass_utils import run_bass_kernel_spmd

    nc = _build()
    in_maps = [make_core_inputs(x, w_attn, w_proj, c) for c in range(8)]
    res = run_bass_kernel_spmd(nc, in_maps, core_ids=list(range(8)))

    y = np.empty((B, T, C), dtype=np.float32)
    for b in range(B):
        yT = (res.results[2 * b]["yT"].astype(np.float32)
              + res.results[2 * b + 1]["yT"].astype(np.float32))
        y[b] = yT.T
    return y
